# revision 22
# baseline (speedup 1.0000x reference)
"""Trainium2 Bass kernel for nn_DecoderAMRPALayer (B=2, S=2048, E=2048, d_k=128).

Sharding: 8 cores = 2 batches x 4 row-chunks of 512 query rows. Each core's
hidden input is row-rotated so its 512 local rows come first; the attention
key/value axis is then a (consistent) permutation of positions, which softmax
and the j-contractions are invariant to.

Folded formulation (weight-weight products hoisted to host):
  scores_raw = hid_loc @ Wqk @ hid^T          with Wqk = Wq @ Wk^T
  logits     = SCALE * (scores_raw + Cb)      Cb = lw*tanh(...)@Kcam^T + u
                                              (CAM bias + query-bias term
                                               u = (bq@Wk^T)@hid^T, host f32)
  A   = softmax(logits)                       (Q/K biases otherwise cancel)
  out = A @ hid @ Wvp + b_out                 with Wvp = Wv @ Wp,
                                              b_out = bv@Wp + bp (A rows sum 1)

Device stages (per core, 512 local queries):
  s2: P^T = Wqk^T @ hidT_loc                  16x[128,512] SBUF tiles
  s7: scores -> +Cb (DVE) -> exp (unnormalized) -> A^T (PE transposes)
      (512-col chunk pipeline: chunk j's exp/flush hides under chunk j+1's
       dense matmuls; per-row sums go to DRAM, host normalizes)
  s8: H^T = hid^T @ Aexp^T                    (streams hid rows)
  s9: outu^T = Wvp^T @ H^T                    -> DRAM [E,512] f32; host does
                                              outu^T.T / rowsum + b_out

All data tensors bf16 (psum f32). PSUM: 7 [128,512] accumulator banks (ring;
with groups of 4 only the earliest-copied bank is shared between consecutive
groups) + 1 transpose bank. DMA schedule (a DMA occupies its issuing engine
~655ns/128KB): sync+scalar carry ONLY the weight streams (16-deep prefetch
rings — any other transfer in those FIFOs head-of-line blocks the tile PE
needs next); hidT local columns ride gpsimd ahead of s2's k-loop demand; all
remaining loads land after the s2 stream, staggered by their s7 deadlines
(s7 is j4-outermost, so hidT chunk c isn't needed until s7start + c*13.8us):
cb + chunk2 on sync, chunk3 on scalar, chunk1 on gpsimd behind a pt[12]
guard; outputs round-robin all three queues.
"""

import sys

sys.path.insert(0, "/opt/trn_rl_repo")

import numpy as np

import concourse.bass as bass
import concourse.mybir as mybir
from concourse import bacc
from concourse.bass import ts
from concourse.bass_utils import run_bass_kernel_spmd
from concourse.masks import make_identity
from concourse.tile import TileContext

F32 = mybir.dt.float32
BF16 = mybir.dt.bfloat16
AF = mybir.ActivationFunctionType
ALU = mybir.AluOpType

S = 2048
E = 2048
LOC = 512  # local query rows per core
DK = 128
NT = E // 128  # 16 partition tiles
SCALE = 1.0 / float(np.sqrt(128.0))
P = 128
WBUFS = 16  # weight-stream prefetch depth
SLOTB = 7  # psum accumulator ring: 7 banks (+1 transpose bank = 8)


def build():
    nc = bacc.Bacc("TRN2", target_bir_lowering=False, debug=False)

    hidT_d = nc.dram_tensor("hidT", [E, S], BF16, kind="ExternalInput").ap()
    hid = nc.dram_tensor("hid", [S, E], BF16, kind="ExternalInput").ap()
    wqk = nc.dram_tensor("wqk", [E, E], BF16, kind="ExternalInput").ap()
    wvp = nc.dram_tensor("wvp", [E, E], BF16, kind="ExternalInput").ap()
    cb_d = nc.dram_tensor("cb", [LOC, S], BF16, kind="ExternalInput").ap()
    out = nc.dram_tensor("out", [E, LOC], F32, kind="ExternalOutput").ap()
    sums_d = nc.dram_tensor("sums", [P, 4], F32, kind="ExternalOutput").ap()

    with TileContext(nc) as tc:
        with (
            tc.tile_pool(name="const", bufs=1) as pconst,
            tc.tile_pool(name="psS", bufs=1, space="PSUM") as psS,
            tc.tile_pool(name="psB", bufs=1, space="PSUM") as psB,
        ):
            ident_f = pconst.tile([P, P], F32, tag="identf")
            ident = pconst.tile([P, P], BF16, tag="ident")
            guard = pconst.tile([1, 1], BF16, tag="guard")
            # PE clock warm-up: ~4us of zero matmuls during the preamble's
            # DMA window so the first real dense block starts at full DVFS
            # pstate (the first ~3us of PE work otherwise run at half clock)
            warm = pconst.tile([P, 512], BF16, tag="warm")
            nc.vector.memset(warm, 0.0)
            wps = psS.tile([P, 512], F32, tag="slot", bufs=SLOTB, name="warm")
            for _ in range(20):
                nc.tensor.matmul(wps, warm[:, 0:P], warm, start=True,
                                 stop=True)

            def mm(ps, lhsT, rhs, start, stop):
                nc.tensor.matmul(ps, lhsT, rhs, start=start, stop=stop)

            def cpy(i, dst, src):
                # all psum->sbuf copies on DVE: ACT issues stream DMAs, and
                # a copy queued behind its DMA backlog stalls the next
                # stage's psum-bank recycling (GpSimd cannot read PSUM)
                nc.vector.tensor_copy(dst, src)

            def slots4():
                return [psS.tile([P, 512], F32, tag="slot", bufs=SLOTB,
                                 name=f"sl{j}") for j in range(4)]

            with tc.tile_pool(name="at", bufs=1) as pat:
                AT = pat.tile([P, NT * 512], BF16, tag="AT")
                sums_sb = pat.tile([P, 4], F32, tag="sums")
                # s8 tiles allocated BEFORE the hidT pool: disjoint SBUF, so
                # the hid stream prefetch needn't wait for s7 to drain
                with tc.tile_pool(name="ht", bufs=1) as pht:
                    ht = [pht.tile([P, LOC], BF16, tag=f"ht{m}", name=f"ht{m}")
                          for m in range(NT)]
                    with tc.tile_pool(name="hidT", bufs=1) as phid:
                        h = [phid.tile([P, S], BF16, tag=f"h{i}", name=f"h{i}")
                             for i in range(NT)]
                        cbt = [phid.tile([P, 512], BF16, tag=f"cb{i}",
                                         name=f"cb{i}") for i in range(16)]
                        # s1: local hidT col-chunk all on gpsimd, in k
                        # order (tile k lands ~8+0.7k us, ahead of s2's
                        # k-loop demand at ~11.6+0.86k us) — sync+scalar
                        # both stay clear to alternate the wqt stream
                        for k in range(NT):
                            nc.gpsimd.dma_start(h[k][:, 0:LOC],
                                                hidT_d[ts(k, P), 0:LOC])
                        # identity for the PE transposes (gpsimd, after the
                        # critical loads)
                        make_identity(nc, ident_f)
                        nc.vector.tensor_copy(ident, ident_f)

                        with tc.tile_pool(name="ppt", bufs=1) as ppt:
                            pt = [ppt.tile([P, LOC], BF16, tag=f"pt{m}",
                                           name=f"pt{m}") for m in range(NT)]

                            # s2: P^T = Wqk^T @ hidT_loc (16x[128,512] bf16)
                            # sync+scalar carry ONLY the wqt stream (any
                            # other transfer in those FIFOs head-of-line
                            # blocks the tile PE needs next)
                            for m4 in range(4):
                                slots = slots4()
                                for k in range(NT):
                                    wqt = ppt.tile([P, 512], BF16, tag="w_in",
                                                   bufs=WBUFS, name="wqt")
                                    (nc.sync, nc.scalar)[k % 2].dma_start(
                                        wqt, wqk[ts(k, P), ts(m4, 512)])
                                    for j in range(4):
                                        mm(slots[j], wqt[:, ts(j, P)],
                                           h[k][:, 0:LOC], k == 0, k == NT - 1)
                                for j in range(4):
                                    cpy(j, pt[m4 * 4 + j], slots[j])

                            # All remaining loads land AFTER the s2 stream
                            # (s7 is j4-outermost, so hidT chunk c is not
                            # needed until s7start + c*13.8us, and cb tile
                            # (ic,j4) not until j4*13.8 + ic*3.46us):
                            #  sync:   cb (j4-major) then hidT chunk2
                            #  scalar: hidT chunk3
                            #  gpsimd: hidT chunk1 behind a pt[12] guard
                            for j4 in range(4):
                                for ic in range(4):
                                    nc.sync.dma_start(
                                        cbt[ic * 4 + j4],
                                        cb_d[ts(ic, P), ts(j4, 512)])
                            for kk in range(16):
                                nc.sync.dma_start(
                                    h[kk][:, ts(2, 512)],
                                    hidT_d[ts(kk, P), ts(2, 512)])
                            for kk in range(16):
                                nc.scalar.dma_start(
                                    h[kk][:, ts(3, 512)],
                                    hidT_d[ts(kk, P), ts(3, 512)])
                            nc.gpsimd.tensor_copy(guard, pt[12][0:1, 0:1])
                            for kk in range(16):
                                nc.gpsimd.dma_start(
                                    h[kk][:, ts(1, 512)],
                                    hidT_d[ts(kk, P), ts(1, 512)])

                            # s7: scores -> +Cb -> exp -> A^T; one [128,512]
                            # psum slot per (key-chunk j4, query-block ic),
                            # j4 outermost; each chunk's exp/flush hides
                            # under the next block's dense matmuls
                            def flush_chunk(ic, jq, src):
                                tp = psB.tile([P, 512], BF16, tag="tr",
                                              bufs=1, name="tp")
                                for t in range(4):
                                    nc.tensor.matmul(
                                        tp[:, ts(t, P)],
                                        src[:, ts(jq * 4 + t, P)], ident,
                                        start=True, stop=True,
                                        is_transpose=True,
                                        skip_group_check=True)
                                nc.vector.tensor_copy(
                                    AT[:, jq * 2048:(jq + 1) * 2048].rearrange(
                                        "p (t i) -> p t i", t=4)[:, :, ts(ic, P)],
                                    tp.rearrange("p (t i) -> p t i", t=4))

                            pending = None  # (ic, jq, src) awaiting flush
                            # exu tiles live in the outer pool: the final
                            # chunk is flushed from inside s8, after
                            # ppt/phid have closed
                            exus = [pat.tile([P, S], BF16, tag=f"exu{ic}",
                                             name=f"exu{ic}")
                                    for ic in range(4)]
                            for j4 in range(4):
                                for ic in range(4):
                                    slot = psS.tile([P, 512], F32, tag="slot",
                                                    bufs=SLOTB, name="sl")
                                    for k in range(NT):
                                        mm(slot, pt[k][:, ts(ic, P)],
                                           h[k][:, ts(j4, 512)],
                                           k == 0, k == NT - 1)
                                    nc.vector.tensor_tensor(
                                        slot, slot, cbt[ic * 4 + j4],
                                        op=ALU.add)
                                    st = ppt.tile([P, 1], F32, tag="st",
                                                  bufs=4, name="st")
                                    nc.scalar.activation(
                                        exus[ic][:, ts(j4, 512)], slot,
                                        AF.Exp, scale=SCALE, accum_out=st)
                                    if j4 == 0:
                                        nc.vector.tensor_copy(
                                            sums_sb[:, ic:ic + 1], st)
                                    else:
                                        nc.vector.tensor_tensor(
                                            sums_sb[:, ic:ic + 1],
                                            sums_sb[:, ic:ic + 1], st,
                                            op=ALU.add)
                                    if pending is not None:
                                        flush_chunk(*pending)
                                    pending = (ic, j4, exus[ic])
                            nc.gpsimd.dma_start(sums_d, sums_sb)

                    # s8: H^T = hid^T @ Aexp^T (streams hid rows)
                    for m4 in range(4):
                        slots = slots4()
                        for k in range(NT):
                            hb = pht.tile([P, 512], BF16, tag="w_in",
                                          bufs=WBUFS, name="hb")
                            (nc.sync, nc.scalar)[k % 2].dma_start(
                                hb, hid[ts(k, P), ts(m4, 512)])
                            for j in range(4):
                                mm(slots[j], hb[:, ts(j, P)],
                                   AT[:, ts(k, 512)], k == 0, k == NT - 1)
                            if m4 == 0 and k == 3 and pending is not None:
                                # final A^T chunk: flushed once its exp has
                                # certainly retired — zero PE wait
                                flush_chunk(*pending)
                                pending = None
                        for j in range(4):
                            cpy(j, ht[m4 * 4 + j], slots[j])

                    with tc.tile_pool(name="cx", bufs=1) as pcx:
                        # s9: outu^T = Wvp^T @ H^T (normalization on host)
                        rings = (nc.sync, nc.scalar, nc.gpsimd)
                        for n4 in range(4):
                            slots = slots4()
                            for k in range(NT):
                                wvt = pcx.tile([P, 512], BF16, tag="w_in",
                                               bufs=WBUFS, name="wvt")
                                (nc.sync, nc.scalar)[k % 2].dma_start(
                                    wvt, wvp[ts(k, P), ts(n4, 512)])
                                for j in range(4):
                                    mm(slots[j], wvt[:, ts(j, P)], ht[k],
                                       k == 0, k == NT - 1)
                            for j in range(4):
                                m = n4 * 4 + j
                                ostg = pcx.tile([P, 512], F32, tag="ostg",
                                                bufs=4, name="ostg")
                                cpy(j, ostg, slots[j])
                                rings[m % 3].dma_start(out[ts(m, P), :], ostg)

    nc.compile()
    return nc


_NC = None


def _get_nc():
    global _NC
    if _NC is None:
        _NC = build()
    return _NC


def make_in_maps(hidden_states, c_attn_w, c_attn_b, c_proj_w, c_proj_b,
                 cam_gate, cam_w0, cam_w1):
    import ml_dtypes
    BF = ml_dtypes.bfloat16

    hs = np.ascontiguousarray(np.asarray(hidden_states, dtype=np.float32))
    W = np.asarray(c_attn_w, dtype=np.float32)
    b = np.asarray(c_attn_b, dtype=np.float32)
    Wp = np.ascontiguousarray(np.asarray(c_proj_w, dtype=np.float32))
    bp = np.asarray(c_proj_b, dtype=np.float32)
    gate = np.ascontiguousarray(np.asarray(cam_gate, dtype=np.float32))
    w0 = float(np.asarray(cam_w0).reshape(-1)[0])
    w1 = float(np.asarray(cam_w1).reshape(-1)[0])

    wq = W[:, :E]
    wk = W[:, E:2 * E]
    wv = W[:, 2 * E:]
    bq, bv = b[:E], b[2 * E:]
    lw = 1.0 / (1.0 + np.exp(-(w0 + w1 * 0.5)))

    # weight-weight folds (input-independent, exact up to f32)
    wqk_b = np.ascontiguousarray(wq @ wk.T).astype(BF)
    wvp_b = np.ascontiguousarray(wv @ Wp).astype(BF)
    b_out = (bv.astype(np.float64) @ Wp.astype(np.float64)
             + bp.astype(np.float64))
    wkbq = wk @ bq  # query-bias row: u_t = hid_t . (Wk bq)

    in_maps = []
    for bi in range(2):
        hb = hs[bi]
        # CAM bias chain (f32, host): biases that cancel in softmax dropped
        Qc = hb @ wq[:, :DK] + bq[:DK]
        Kc = hb @ wk[:, :DK]
        Vc = hb @ wv[:, :DK] + bv[:DK]
        bl = (Qc @ Kc.T) * SCALE
        bl -= bl.max(axis=1, keepdims=True)
        eA = np.exp(bl)
        baseA = eA / eA.sum(axis=1, keepdims=True)
        Tm = np.tanh((baseA @ Vc) * gate)
        Cb = lw * (Tm @ Kc.T) + (hb @ wkbq)[None, :]  # [S, S]
        for rr in range(4):
            sel = np.concatenate([np.arange(rr * LOC, S),
                                  np.arange(0, rr * LOC)])
            hid_roll = np.ascontiguousarray(hb[sel]).astype(BF)
            hidT_roll = np.ascontiguousarray(hb[sel].T).astype(BF)
            cb_core = np.ascontiguousarray(
                Cb[rr * LOC:(rr + 1) * LOC][:, sel]).astype(BF)
            in_maps.append({
                "hid": hid_roll, "hidT": hidT_roll, "wqk": wqk_b,
                "wvp": wvp_b, "cb": cb_core,
            })
    return in_maps, b_out


def kernel(**inputs):
    nc = _get_nc()
    in_maps, b_out = make_in_maps(**inputs)
    res = run_bass_kernel_spmd(nc, in_maps, core_ids=list(range(8)))
    out = np.empty((2, S, E), dtype=np.float32)
    for c in range(8):
        bi, rr = divmod(c, 4)
        outu = res.results[c]["out"].astype(np.float64)  # [E, LOC] unnorm^T
        rowsum = res.results[c]["sums"].astype(np.float64).T.reshape(LOC)
        out[bi, rr * LOC:(rr + 1) * LOC] = \
            (outu.T / rowsum[:, None] + b_out).astype(np.float32)
    return out
